# revision 6
# baseline (speedup 1.0000x reference)
"""NT-Xent loss on 8 Trainium2 NeuronCores (Bass/Tile).

Reference computation (B=4096, D=1024, T=0.5):
    x  = concat(z_i, z_j)                      # [8192, 1024] f32
    xn = x / ||x||                             # row-normalize
    sim = xn @ xn.T                            # [8192, 8192]
    logits = sim / T, diag masked to -inf
    loss = -mean(log_softmax(logits)[i, target(i)]), target(i) = i ^ 1

Sharding: row-block parallel. Core c owns rows [1024c, 1024(c+1)). Each
core receives the full x pre-transposed and column-rotated so its own
block sits at rotated columns [0, 1024):
    xt_c[d, n] = x[(n + 1024 c) mod 8192, d]   # [1024, 8192] f32
(Rotation makes the diagonal/target positions identical on every core, so
one SPMD program works for all 8 cores; softmax sums are permutation
invariant.)

On-device per core:
  Phase A (per 512-col chunk): sq-norms via DVE square + PE ones-matmul
  (cross-partition sum), inv = exp(-0.5 ln s) on ACT (Exp/Ln share one
  table set; ACT Rsqrt is banned), PE K=1 broadcast of inv, DVE scale to
  bf16 -> xn^T resident in SBUF (16 MB).
  Phase B (n-outer): per (n-chunk j, m-tile) accumulate 8 K-tile bf16
  matmuls into PSUM [128, 512]; extract diag/target via mask
  tensor_tensor_reduce (only j < 2 after rotation); ACT exp(2*sim)
  in-place on PSUM with accum_out giving per-row partial sums.
  Tail: S - exp(2 diag), lse = ln, row loss = lse - 2*target.
Host: sum the 8x[128, 8] partials, divide by 8192.
"""

import numpy as np
from contextlib import ExitStack

import concourse.bass as bass
import concourse.tile as tile
from concourse import bacc, mybir
from concourse.bass_utils import run_bass_kernel_spmd

F32 = mybir.dt.float32
BF16 = mybir.dt.bfloat16

B = 4096
D = 1024
N = 2 * B            # 8192 rows total
NCORES = 8
RPC = N // NCORES    # 1024 rows per core
KT = D // 128        # 8 contraction partition-tiles
MT = RPC // 128      # 8 row tiles per core
CHUNK = 512
NCH = N // CHUNK     # 16 column chunks

_NC_CACHE = {}
LAST_RESULTS = None  # BassKernelResults of the most recent run (for test.py)


def _build_program():
    nc = bacc.Bacc("TRN2", target_bir_lowering=False, debug=False)

    xt = nc.dram_tensor("xt", [D, N], F32, kind="ExternalInput")
    masks = nc.dram_tensor("masks", [128, 256], F32, kind="ExternalInput")
    loss_out = nc.dram_tensor("loss_parts", [128, MT], F32, kind="ExternalOutput")

    ADD = mybir.AluOpType.add
    MULT = mybir.AluOpType.mult
    EXP = mybir.ActivationFunctionType.Exp
    LN = mybir.ActivationFunctionType.Ln

    with tile.TileContext(nc) as tc, ExitStack() as ctx:
        consts = ctx.enter_context(tc.tile_pool(name="consts", bufs=1))
        xn_pool = ctx.enter_context(tc.tile_pool(name="xn", bufs=1))
        raw_pool = ctx.enter_context(tc.tile_pool(name="raw", bufs=2))
        sq_pool = ctx.enter_context(tc.tile_pool(name="sq", bufs=3))
        inv_pool = ctx.enter_context(tc.tile_pool(name="inv", bufs=2))
        exp_pool = ctx.enter_context(tc.tile_pool(name="exp", bufs=3))
        scr_pool = ctx.enter_context(tc.tile_pool(name="scr", bufs=2))
        stat_pool = ctx.enter_context(tc.tile_pool(name="stat", bufs=1))
        small_pool = ctx.enter_context(tc.tile_pool(name="small", bufs=4))
        ps_s = ctx.enter_context(tc.tile_pool(name="ps_s", bufs=2, space="PSUM"))
        ps_b = ctx.enter_context(tc.tile_pool(name="ps_b", bufs=2, space="PSUM"))
        ps_g = ctx.enter_context(tc.tile_pool(name="ps_g", bufs=4, space="PSUM"))

        mask_sb = consts.tile([128, 256], F32)
        nc.sync.dma_start(mask_sb[:], masks[:])
        ones_km = consts.tile([128, 1], F32)
        nc.vector.memset(ones_km[:], 1.0)
        ones_k1 = consts.tile([1, 128], F32)
        nc.vector.memset(ones_k1[:], 1.0)

        # Full normalized transposed x, bf16, resident: 128 KB/partition.
        xn = xn_pool.tile([128, KT, N], BF16)

        esum = stat_pool.tile([128, MT, NCH], F32)
        ediag = stat_pool.tile([128, MT], F32)
        etarg = stat_pool.tile([128, MT], F32)
        loss_sb = stat_pool.tile([128, MT], F32)

        xt_r = xt[:].rearrange("(k p) n -> p k n", k=KT)

        def phase_a(j):
            """Normalize columns [512j, 512j+512) into xn."""
            csl = slice(CHUNK * j, CHUNK * (j + 1))
            raw = raw_pool.tile([128, KT, CHUNK], F32)
            nc.sync.dma_start(raw[:], xt_r[:, :, csl])
            s_ps = ps_s.tile([1, CHUNK], F32)
            for k in range(KT):
                sq = sq_pool.tile([128, CHUNK], F32)
                nc.vector.tensor_mul(sq[:], raw[:, k, :], raw[:, k, :])
                nc.tensor.matmul(
                    s_ps[:], lhsT=ones_km[:], rhs=sq[:],
                    start=(k == 0), stop=(k == KT - 1),
                )
            lg = inv_pool.tile([1, CHUNK], F32)
            nc.scalar.activation(lg[:], s_ps[:], LN)
            inv = inv_pool.tile([1, CHUNK], F32)
            nc.scalar.activation(inv[:], lg[:], EXP, scale=-0.5)
            b_ps = ps_b.tile([128, CHUNK], F32)
            nc.tensor.matmul(b_ps[:], lhsT=ones_k1[:], rhs=inv[:], start=True, stop=True)
            for k in range(KT):
                nc.vector.tensor_mul(xn[:, k, csl], raw[:, k, :], b_ps[:])

        def sweep(j):
            """All m-tiles against column chunk j; fused softmax stats.

            ACT exp writes SBUF f32 (never in-place PSUM: DVE reads of the
            PSUM g tile alongside the ACT write trip the fatal PSUM
            single-port bank conflict on HW). diag/target are extracted
            from the exp values: ediag is subtracted from the row sum
            as-is, etarg goes back through Ln in the tail.
            """
            csl = slice(CHUNK * j, CHUNK * (j + 1))
            for m in range(MT):
                g = ps_g.tile([128, CHUNK], F32)
                for k in range(KT):
                    nc.tensor.matmul(
                        g[:], lhsT=xn[:, k, 128 * m:128 * (m + 1)],
                        rhs=xn[:, k, csl],
                        start=(k == 0), stop=(k == KT - 1),
                    )
                esb = exp_pool.tile([128, CHUNK], F32)
                nc.scalar.activation(
                    esb[:], g[:], EXP, scale=2.0, accum_out=esum[:, m, j:j + 1],
                )
                if j == m // 4:
                    off = (m % 4) * 128
                    scr = scr_pool.tile([128, 128], F32)
                    nc.vector.tensor_mul(
                        scr[:], esb[:, off:off + 128], mask_sb[:, 0:128])
                    nc.vector.tensor_reduce(
                        ediag[:, m:m + 1], scr[:],
                        axis=mybir.AxisListType.X, op=ADD)
                    scr2 = scr_pool.tile([128, 128], F32)
                    nc.vector.tensor_mul(
                        scr2[:], esb[:, off:off + 128], mask_sb[:, 128:256])
                    nc.vector.tensor_reduce(
                        etarg[:, m:m + 1], scr2[:],
                        axis=mybir.AxisListType.X, op=ADD)

        phase_a(0)
        phase_a(1)
        for j in range(NCH):
            sweep(j)
            if j + 2 < NCH:
                phase_a(j + 2)

        for m in range(MT):
            s_tot = small_pool.tile([128, 1], F32)
            nc.vector.tensor_reduce(
                s_tot[:], esum[:, m, :], axis=mybir.AxisListType.X, op=ADD,
            )
            den = small_pool.tile([128, 1], F32)
            nc.vector.tensor_sub(den[:], s_tot[:], ediag[:, m:m + 1])
            lse = small_pool.tile([128, 1], F32)
            nc.scalar.activation(lse[:], den[:], LN)
            ltarg = small_pool.tile([128, 1], F32)
            nc.scalar.activation(ltarg[:], etarg[:, m:m + 1], LN)
            # loss = lse - ln(exp(2*targ)) = lse - 2*targ
            nc.vector.tensor_sub(loss_sb[:, m:m + 1], lse[:], ltarg[:])
        nc.sync.dma_start(loss_out[:], loss_sb[:])

    nc.finalize()
    return nc


def _get_program():
    if "nc" not in _NC_CACHE:
        _NC_CACHE["nc"] = _build_program()
    return _NC_CACHE["nc"]


def _make_masks():
    m = np.zeros((128, 256), dtype=np.float32)
    p = np.arange(128)
    m[p, p] = 1.0          # identity: diagonal extraction
    m[p, 128 + (p ^ 1)] = 1.0  # pair-swap: target extraction
    return m


def kernel(z_i: np.ndarray, z_j: np.ndarray, _trace: bool = False) -> np.ndarray:
    global LAST_RESULTS
    nc = _get_program()

    x = np.concatenate([np.asarray(z_i), np.asarray(z_j)], axis=0)
    assert x.shape == (N, D) and x.dtype == np.float32
    xT = np.ascontiguousarray(x.T)  # [D, N]
    masks = _make_masks()

    in_maps = []
    for c in range(NCORES):
        xt_c = np.roll(xT, -RPC * c, axis=1)
        in_maps.append({"xt": np.ascontiguousarray(xt_c), "masks": masks})

    res = run_bass_kernel_spmd(
        nc, in_maps, core_ids=list(range(NCORES)), trace=_trace,
    )
    LAST_RESULTS = res

    total = np.float64(0.0)
    for c in range(NCORES):
        total += res.results[c]["loss_parts"].astype(np.float64).sum()
    return np.float32(total / N)
